# revision 15
# baseline (speedup 1.0000x reference)
"""Trainium2 Bass kernel for BoxMultiHeadedAttention (B=8, N=512, D=512, H=8).

Sharding: data-parallel over batch — each of the 8 NeuronCores computes one
batch element end-to-end; weights replicated; no collectives.

v3 schedule (engine-balanced):
  * host pre-transposes inputs/weights to bf16; all matrix consts are
    host-packed bf16 so no on-chip converts are needed.
  * projections: bias applied via a rank-1 ones-matmul appended to each
    chain; PSUM eviction on ACT (Copy). Chains are interleaved with ph4
    rb-blocks in PE program order to fill PE gaps.
  * ln fields (ph2): ACT Square(+bias)/Ln, Pool subtract/clip; then
    bf16 hi/lo split of dxy2 (ACT copy + Pool subtract) so the phase
    selector runs as 3 bf16 matmuls instead of 1 fp32 matmul.
  * wg (ph4): DVE does the magic-round fold (rr/ff) and the double-angle
    products; ACT does the Sin pair and the Relu eviction
    (wgdR = Relu(wg + bG' - eps), with the -1+eps constant folded into a
    precomputed objc tile); Pool holds lhs_wh (all precomputed early).
  * wgd h-major -> m-major shuffle via DRAM bounce (8 per-h writes + 1
    read per rb); uu = objc + objpair*wgdR computed on Pool per rb.
  * attention (ph5): st2 (PE) -> Exp (ACT) -> tt = e*uu (DVE) ->
    AV/rowsum (PE, accumulating); softmax 1/s broadcast across partitions
    with a one-hot matmul; final linear with bias-matmul + ACT evict.
"""
import math
import numpy as np
from contextlib import ExitStack

import concourse.bass as bass
import concourse.mybir as mybir
import concourse.tile as tile
from concourse.bass_utils import run_bass_kernel_spmd

F32 = mybir.dt.float32
BF16 = mybir.dt.bfloat16
AF = mybir.ActivationFunctionType
ALU = mybir.AluOpType

B, N, D, H = 8, 512, 512, 8
DK = D // H
P = 128
NRB = N // P
NG = 8
GM = 16
NPAIR = H // 2
WAVE_LEN = 1000.0
MAGIC = 12582912.0
C2 = float(2.0 * math.log(0.001))
ESHIFT = -6.0
CM1 = 1e-6 - 1.0
TWO_PI = float(2.0 * math.pi)
HALF_PI = float(math.pi / 2.0)
PI_ = float(math.pi)

_alphas = (100.0 / (WAVE_LEN ** (np.arange(8) / 8.0))).astype(np.float64)

# column indices in colpack
C_BQ, C_BK, C_MC, C_CX, C_CY, C_NCX, C_NCY, C_OC = 0, 4, 8, 12, 16, 20, 24, 28
C_BGR, C_HPI = 32, 33
NCOL = 34
# row indices in rowg
R_CX, R_CY, R_LW, R_LH = range(4)
NROWG = 4
# rows in browpack
BR_Q, BR_K, BR_V, BR_O = range(4)


def _split_multi_waits(nc):
    """walrus here accepts only ONE sync-wait per ISA instruction; hoist
    extras onto NoOps inserted before the offending instruction."""
    n_fix = 0
    for blk in nc.main_func.blocks:
        insts = list(blk.instructions)
        out, dirty = [], False
        for inst in insts:
            si = inst.sync_info
            waits = list(si.on_wait) if si is not None else []
            if len(waits) > 1:
                for kk, w in enumerate(waits[:-1]):
                    out.append(mybir.InstNoOp(
                        name=f"I-waitfix-{n_fix}-{kk}", engine=inst.engine,
                        sync_info=mybir.SyncInfo(on_wait=[w], on_update=[])))
                inst.sync_info = mybir.SyncInfo(
                    on_wait=[waits[-1]], on_update=list(si.on_update))
                n_fix += 1
                dirty = True
            out.append(inst)
        if dirty:
            blk.instructions = out
    return n_fix


def _selector_const():
    # SELAP[64*W + q*16 + m_loc, q, m_loc*8 + j] = alpha_j/(4pi)
    selap = np.zeros((P, 4, P), dtype=np.float32)
    for W in range(2):
        for q in range(4):
            for m_loc in range(GM):
                for j in range(8):
                    selap[64 * W + q * 16 + m_loc, q, m_loc * 8 + j] = \
                        _alphas[j] / (4.0 * math.pi)
    return selap


def _onehot2():
    oh = np.zeros((P, 2, 2), dtype=np.float32)
    for hi in range(2):
        oh[:, hi, hi] = 1.0
    return oh


def _selpair():
    sp = np.zeros((2, P), dtype=np.float32)
    for hi in range(2):
        sp[hi, hi * DK:(hi + 1) * DK] = 1.0
    return sp


def _wg_consts(WG, bG):
    out = {}
    gmap = [lambda j: j, lambda j: 32 + j, lambda j: 8 + j, lambda j: 40 + j]
    gscl = [2.0, -2.0, 2.0, -2.0]
    wblk = np.zeros((4, P, P), dtype=np.float32)
    for c in range(4):
        for m_loc in range(GM):
            for j in range(8):
                for h in range(H):
                    wblk[c, m_loc * 8 + j, h * GM + m_loc] = \
                        gscl[c] * WG[h, gmap[c](j)]
    out["WBLK"] = wblk.transpose(1, 0, 2).copy()  # [P, 4, P]

    acol = np.zeros((64, 1), np.float32)
    pcol_m = np.zeros((64, 1), np.float32)
    pcol_n = np.zeros((64, 1), np.float32)
    w1 = np.zeros((64, H), np.float32)
    for f in range(2):
        for j in range(8):
            gs = 16 + 8 * f + j
            gc = 48 + 8 * f + j
            a = _alphas[j] / (4.0 * math.pi)
            for t in range(4):
                k = (f * 8 + j) * 4 + t
                acol[k, 0] = a
                pcol_m[k, 0] = 0.25 if t in (0, 2) else 0.0
                if t == 0:
                    pcol_n[k, 0] = 0.0; w1[k] = WG[:, gs]
                elif t == 1:
                    pcol_n[k, 0] = 0.75; w1[k] = WG[:, gs]   # -cos -> +pi
                elif t == 2:
                    pcol_n[k, 0] = 0.25; w1[k] = WG[:, gc]
                else:
                    pcol_n[k, 0] = 0.0; w1[k] = WG[:, gc]
    out["ACOL"] = acol
    out["PCOL_M"], out["PCOL_N"] = pcol_m, pcol_n
    out["W1E"] = np.repeat(w1, GM, axis=1).astype(np.float32)
    bg2 = bG.astype(np.float64) + WG[:, 32:48].sum(axis=1)
    out["BGR"] = np.repeat((bg2 - 1e-6).astype(np.float32), GM)
    return out


def _to_bf16(a):
    import ml_dtypes
    return np.asarray(a, np.float32).astype(ml_dtypes.bfloat16)


def _host_prep(inputs):
    q = np.asarray(inputs["input_query"], np.float32)
    k = np.asarray(inputs["input_key"], np.float32)
    v = np.asarray(inputs["input_value"], np.float32)
    box = np.asarray(inputs["input_box"], np.float32)
    mask = np.asarray(inputs["mask"])
    nobj = np.asarray(inputs["not_objects"])
    WG = np.asarray(inputs["WG"], np.float32)
    bG = np.asarray(inputs["bG"], np.float32)
    wgc = _wg_consts(WG, bG)
    sela = _selector_const()
    selah = _to_bf16(sela)
    selal = _to_bf16(sela - np.asarray(selah, np.float32))
    selhl = np.concatenate([np.asarray(selah, np.float32)[:, None],
                            np.asarray(selal, np.float32)[:, None]],
                           axis=1)  # [P, 2, 4, P]

    x_min, y_min, x_max, y_max = [box[..., i] for i in range(4)]
    cx = (x_min + x_max) * 0.5
    cy = (y_min + y_max) * 0.5
    ww = x_max - x_min + 1.0
    hh = y_max - y_min + 1.0
    l2w = (2.0 * np.log(ww)).astype(np.float32)
    l2h = (2.0 * np.log(hh)).astype(np.float32)

    maskcol = (np.where(mask == 0, -1e9, 0.0) + ESHIFT).astype(np.float32)
    obj = (1.0 - nobj.astype(np.float32)).astype(np.float32)

    def wtile(W, scale=1.0):
        return _to_bf16((np.asarray(W, np.float32) * scale)
                        .reshape(NRB, P, D).transpose(1, 0, 2).copy())

    def xtile(x):
        return _to_bf16(x.T.reshape(NRB, P, N).transpose(1, 0, 2).copy())

    w64 = np.zeros((64, 131), np.float32)
    w64[:, :128] = wgc["W1E"]
    w64[:, 128] = wgc["ACOL"][:, 0]
    w64[:, 129] = wgc["PCOL_M"][:, 0]
    w64[:, 130] = wgc["PCOL_N"][:, 0]

    brow = np.zeros((1, 4, D), np.float32)
    brow[0, BR_Q] = np.asarray(inputs["bq"], np.float32)
    brow[0, BR_K] = np.asarray(inputs["bk"], np.float32) * 0.125
    brow[0, BR_V] = np.asarray(inputs["bv"], np.float32)
    brow[0, BR_O] = np.asarray(inputs["bo"], np.float32)

    shared = {
        "Wqb": wtile(inputs["Wq"]),
        "Wkb": wtile(inputs["Wk"], 0.125),
        "Wvb": wtile(inputs["Wv"]),
        "Wob": wtile(inputs["Wo"]),
        "SELHL": _to_bf16(selhl),
        "WBLK": _to_bf16(wgc["WBLK"]),
        "W64": w64,
        "OH2": _to_bf16(_onehot2()),
        "SELP": _selpair(),
        "BROW": _to_bf16(brow),
    }
    in_maps = []
    for b in range(B):
        colpack = np.zeros((P, NCOL), np.float32)
        colpack[:, C_BQ:C_BQ + 4] = np.asarray(inputs["bq"], np.float32) \
            .reshape(NRB, P).T
        colpack[:, C_BK:C_BK + 4] = (np.asarray(inputs["bk"], np.float32)
                                     * 0.125).reshape(NRB, P).T
        colpack[:, C_MC:C_MC + 4] = maskcol[b].reshape(NRB, P).T
        colpack[:, C_CX:C_CX + 4] = cx[b].reshape(NRB, P).T
        colpack[:, C_CY:C_CY + 4] = cy[b].reshape(NRB, P).T
        colpack[:, C_NCX:C_NCX + 4] = -cx[b].reshape(NRB, P).T
        colpack[:, C_NCY:C_NCY + 4] = -cy[b].reshape(NRB, P).T
        colpack[:, C_OC:C_OC + 4] = obj[b].reshape(NRB, P).T
        colpack[:, C_BGR] = wgc["BGR"]
        colpack[:, C_HPI] = HALF_PI

        rowg = np.zeros((NROWG, N), np.float32)
        rowg[R_CX] = cx[b]
        rowg[R_CY] = cy[b]
        rowg[R_LW] = l2w[b]
        rowg[R_LH] = l2h[b]

        m = dict(shared)
        m.update({
            "xqT": xtile(q[b]), "xkT": xtile(k[b]), "xvT": xtile(v[b]),
            "colpack": colpack, "rowg": rowg,
            "objrow": obj[b].copy(),
        })
        in_maps.append(m)
    return in_maps


def build_nc():
    nc = bass.Bass()

    def dp(name, shape, dt=F32):
        return nc.declare_dram_parameter(name, list(shape), dt, isOutput=False)

    colpack = dp("colpack", (P, NCOL))
    rowg = dp("rowg", (NROWG, N))
    objrow = dp("objrow", (N,))
    xqT = dp("xqT", (P, NRB, N), BF16)
    xkT = dp("xkT", (P, NRB, N), BF16)
    xvT = dp("xvT", (P, NRB, N), BF16)
    Wqb = dp("Wqb", (P, NRB, D), BF16)
    Wkb = dp("Wkb", (P, NRB, D), BF16)
    Wvb = dp("Wvb", (P, NRB, D), BF16)
    Wob = dp("Wob", (P, NRB, D), BF16)
    SELHL = dp("SELHL", (P, 2, 4, P), BF16)
    WBLK = dp("WBLK", (P, 4, P), BF16)
    W64 = dp("W64", (64, 131))
    OH2 = dp("OH2", (P, 2, 2), BF16)
    SELP = dp("SELP", (2, P))
    BROW = dp("BROW", (1, 4, D), BF16)
    out = nc.declare_dram_parameter("out", [N, D], F32, isOutput=True)
    wgdd = nc.dram_tensor("wgdd", [NRB, NG, GM, H, N], BF16)

    with ExitStack() as ctx:
        tc = ctx.enter_context(tile.TileContext(nc))
        const = ctx.enter_context(tc.tile_pool(name="const", bufs=1))
        persist = ctx.enter_context(tc.tile_pool(name="persist", bufs=1))

        # ---------------- loads (critical geo consts first) ----------------
        col_t = const.tile([P, NCOL], F32, tag="colpk")
        nc.sync.dma_start(col_t[:], colpack[:])
        rowbc = const.tile([P, NROWG, N], F32, tag="rowg")
        nc.sync.dma_start(
            rowbc[:], rowg[None, :, :].to_broadcast((P, NROWG, N)))
        w64_t = const.tile([64, 131], F32, tag="w64")
        nc.sync.dma_start(w64_t[:], W64[:])
        selhl_t = const.tile([P, 2, 4, P], BF16, tag="selhl")
        nc.sync.dma_start(selhl_t[:], SELHL[:])

        xqb = persist.tile([P, NRB, N], BF16, tag="xqb")
        nc.sync.dma_start(xqb[:], xqT[:])
        wq_b = persist.tile([P, NRB, D], BF16, tag="wqb")
        nc.sync.dma_start(wq_b[:], Wqb[:])
        xkb = persist.tile([P, NRB, N], BF16, tag="xkb")
        nc.sync.dma_start(xkb[:], xkT[:])
        wk_b = persist.tile([P, NRB, D], BF16, tag="wkb")
        nc.sync.dma_start(wk_b[:], Wkb[:])
        xvb = persist.tile([P, NRB, N], BF16, tag="xvb")
        nc.sync.dma_start(xvb[:], xvT[:])
        wv_b = persist.tile([P, NRB, D], BF16, tag="wvb")
        nc.sync.dma_start(wv_b[:], Wvb[:])

        wblk_b = const.tile([P, 4, P], BF16, tag="wblkb")
        nc.sync.dma_start(wblk_b[:], WBLK[:])
        objbc_f = const.tile([P, N], F32, tag="objbcf")
        nc.sync.dma_start(objbc_f[:], objrow[None, :].to_broadcast((P, N)))
        oh2_t = const.tile([P, 2, 2], BF16, tag="oh2")
        nc.sync.dma_start(oh2_t[:], OH2[:])
        selp_f = const.tile([2, P], F32, tag="selpf")
        nc.sync.dma_start(selp_f[:], SELP[:])
        brow_t = const.tile([1, 4, D], BF16, tag="brow")
        nc.sync.dma_start(brow_t[:], BROW[:])
        wo_b = persist.tile([P, NRB, D], BF16, tag="wob")
        nc.sync.dma_start(wo_b[:], Wob[:])

        ones_row = const.tile([1, N], BF16, tag="ones_row")
        nc.vector.memset(ones_row[:], 1.0)
        objbc = const.tile([P, N], BF16, tag="objbc")
        nc.gpsimd.tensor_copy(objbc[:], objbc_f[:])

        w1e_f = w64_t[:, 0:128]
        acol_t = w64_t[:, 128:129]
        pcolm_t = w64_t[:, 129:130]
        pcoln_t = w64_t[:, 130:131]

        # ---------------- ph2: ln fields + hi/lo split ----------------
        dxyh = persist.tile([P, NRB, 2, N], BF16, tag="dxyh")
        dxyl = persist.tile([P, NRB, 2, N], BF16, tag="dxyl")
        with tc.tile_pool(name="work2", bufs=2) as work2:
            for rb in range(NRB):
                dxy2 = work2.tile([P, 2, N], F32, tag="dxy2")
                for (ci, rbc, ncc) in ((0, R_CX, C_NCX), (1, R_CY, C_NCY)):
                    d2 = work2.tile([P, N], F32, tag="geo_d2")
                    nc.scalar.activation(d2[:], rowbc[:, rbc, :], AF.Square,
                                         bias=col_t[:, ncc + rb:ncc + rb + 1])
                    l2t = work2.tile([P, N], F32, tag="geo_l2")
                    nc.scalar.activation(l2t[:], d2[:], AF.Ln)
                    g_ = work2.tile([P, N], F32, tag="geo_g")
                    nc.gpsimd.tensor_tensor(
                        g_[:], l2t[:], rowbc[:, R_LW + ci, :], ALU.subtract)
                    nc.gpsimd.tensor_scalar(dxy2[:, ci, :], g_[:],
                                            C2, None, ALU.max)
                nc.scalar.activation(dxyh[:, rb, :, :], dxy2[:], AF.Copy)
                nc.gpsimd.tensor_tensor(dxyl[:, rb, :, :], dxy2[:],
                                        dxyh[:, rb, :, :], ALU.subtract)

        # ---------------- ph3: dw/dh banks (Pool + ACT) ----------------
        bankM = persist.tile([64, N], BF16, tag="bankM")
        bankN = persist.tile([64, N], BF16, tag="bankN")
        with tc.tile_pool(name="work3", bufs=2) as work3:
            for (pcol, bank) in ((pcolm_t, bankM), (pcoln_t, bankN)):
                t_ = work3.tile([64, N], F32, tag="bk_t")
                nc.gpsimd.tensor_scalar(t_[:32, :], rowbc[:32, R_LW, :],
                                        acol_t[:32, :], pcol[:32, :],
                                        ALU.mult, ALU.add)
                nc.gpsimd.tensor_scalar(t_[32:, :], rowbc[32:64, R_LH, :],
                                        acol_t[32:, :], pcol[32:, :],
                                        ALU.mult, ALU.add)
                r_ = work3.tile([64, N], F32, tag="bk_r")
                nc.gpsimd.tensor_scalar(r_[:], t_[:], MAGIC, -MAGIC,
                                        ALU.add, ALU.add)
                f_ = work3.tile([64, N], F32, tag="bk_f")
                nc.gpsimd.tensor_tensor(f_[:], t_[:], r_[:], ALU.subtract)
                nc.scalar.activation(bank[:], f_[:], AF.Sin, scale=TWO_PI)

        # lhs_wh precompute (Pool): [64, rb, g, P]
        lhs_all = persist.tile([64, NRB, NG, P], BF16, tag="lhs_all")
        for rb in range(NRB):
            for g in range(NG):
                mbase = rb * P + g * GM
                nc.gpsimd.tensor_tensor(
                    lhs_all[:, rb, g, :].rearrange("k (h m) -> k h m", h=H),
                    w1e_f.rearrange("k (h m) -> k h m", h=H),
                    bankM[:, mbase:mbase + GM][:, None, :]
                        .to_broadcast((64, H, GM)),
                    ALU.mult)

        # objpair/objc per rb (Pool)
        objpair = persist.tile([P, NRB, N], BF16, tag="objpair")
        objc = persist.tile([P, NRB, N], BF16, tag="objc")
        for rb in range(NRB):
            nc.gpsimd.tensor_scalar(objpair[:, rb, :], objbc[:],
                                    col_t[:, C_OC + rb:C_OC + rb + 1], None,
                                    ALU.mult)
            nc.gpsimd.tensor_scalar(objc[:, rb, :], objpair[:, rb, :],
                                    CM1, 1.0, ALU.mult, ALU.add)

        # ---------------- ph1 chains + ph4, PE-interleaved ----------------
        qT = persist.tile([P, NRB, N], BF16, tag="qT")
        kTt = persist.tile([P, NRB, N], BF16, tag="kT")
        v_sb = persist.tile([P, NRB, D], BF16, tag="v_sb")
        uu_all = persist.tile([P, NPAIR, NRB, 2, N], BF16, tag="uu_all")

        with tc.tile_pool(name="work4", bufs=2) as work4, \
             tc.tile_pool(name="ilpool", bufs=2) as ilpool, \
             tc.tile_pool(name="wstp", bufs=2) as wstp, \
             tc.tile_pool(name="psum1", bufs=2, space="PSUM") as psum1, \
             tc.tile_pool(name="psum_u", bufs=2, space="PSUM") as psum_u, \
             tc.tile_pool(name="psum_wg", bufs=2, space="PSUM") as psum_wg:

            def qk_chain(ob):
                for (wb_, xb, dstT, br) in ((wq_b, xqb, qT, BR_Q),
                                            (wk_b, xkb, kTt, BR_K)):
                    ps = psum1.tile([P, N], F32, tag="projps")
                    for kb in range(NRB):
                        nc.tensor.matmul(ps[:],
                                         wb_[:, kb, ob * P:(ob + 1) * P],
                                         xb[:, kb, :],
                                         start=(kb == 0), stop=False)
                    nc.tensor.matmul(ps[:],
                                     brow_t[0:1, br, ob * P:(ob + 1) * P],
                                     ones_row[:],
                                     start=False, stop=True)
                    nc.scalar.activation(dstT[:, ob, :], ps[:], AF.Copy)

            def v_chain(mb):
                ps = psum1.tile([P, D], F32, tag="projps")
                for kb in range(NRB):
                    nc.tensor.matmul(ps[:], xvb[:, kb, mb * P:(mb + 1) * P],
                                     wv_b[:, kb, :],
                                     start=(kb == 0), stop=False)
                nc.tensor.matmul(ps[:], ones_row[0:1, mb * P:(mb + 1) * P],
                                 brow_t[0:1, BR_V, :], start=False, stop=True)
                nc.scalar.activation(v_sb[:, mb, :], ps[:], AF.Copy)

            for rb in range(NRB):
                # interleave projection chains into PE gaps
                if rb < 2:
                    qk_chain(rb)
                elif rb == 2:
                    qk_chain(2)
                    qk_chain(3)
                else:
                    for mb in range(NRB):
                        v_chain(mb)

                wgd_il = ilpool.tile([P, NG, N], BF16, tag="wgd_il")
                for g in range(NG):
                    ups = psum_u.tile([P, 2, N], F32, tag="ups")
                    off = 64 * (g // 4)
                    qq = g % 4
                    for ci in range(2):
                        nc.tensor.matmul(ups[:, ci, :],
                                         selhl_t[off:off + 64, 0, qq, :],
                                         dxyh[off:off + 64, rb, ci, :],
                                         start=True, stop=False)
                        nc.tensor.matmul(ups[:, ci, :],
                                         selhl_t[off:off + 64, 0, qq, :],
                                         dxyl[off:off + 64, rb, ci, :],
                                         start=False, stop=False)
                        nc.tensor.matmul(ups[:, ci, :],
                                         selhl_t[off:off + 64, 1, qq, :],
                                         dxyh[off:off + 64, rb, ci, :],
                                         start=False, stop=True)
                    rr = work4.tile([P, 2, N], F32, tag="fold_r")
                    nc.vector.tensor_scalar(rr[:], ups[:], MAGIC, -MAGIC,
                                            ALU.add, ALU.add)
                    ff = work4.tile([P, 2, N], F32, tag="fold_f")
                    nc.vector.tensor_tensor(ff[:], ups[:], rr[:],
                                            ALU.subtract)
                    s2 = work4.tile([P, 2, N], BF16, tag="s2")
                    nc.scalar.activation(s2[:], ff[:], AF.Sin, scale=PI_)
                    c2 = work4.tile([P, 2, N], BF16, tag="c2")
                    nc.scalar.activation(c2[:], ff[:], AF.Sin, scale=-PI_,
                                         bias=col_t[:, C_HPI:C_HPI + 1])
                    fsin = work4.tile([P, 2, N], BF16, tag="fsin")
                    nc.vector.tensor_tensor(fsin[:], s2[:], c2[:], ALU.mult)
                    fcos = work4.tile([P, 2, N], BF16, tag="fcos")
                    nc.vector.tensor_tensor(fcos[:], s2[:], s2[:], ALU.mult)
                    wgp = psum_wg.tile([P, N], F32, tag="wgp")
                    nc.tensor.matmul(wgp[:], wblk_b[:, 0, :], fsin[:, 0, :],
                                     start=True, stop=False)
                    nc.tensor.matmul(wgp[:], wblk_b[:, 1, :], fcos[:, 0, :],
                                     start=False, stop=False)
                    nc.tensor.matmul(wgp[:], wblk_b[:, 2, :], fsin[:, 1, :],
                                     start=False, stop=False)
                    nc.tensor.matmul(wgp[:], wblk_b[:, 3, :], fcos[:, 1, :],
                                     start=False, stop=False)
                    nc.tensor.matmul(wgp[:], lhs_all[:, rb, g, :], bankN[:],
                                     start=False, stop=True)
                    nc.scalar.activation(wgd_il[:, g, :], wgp[:], AF.Relu,
                                         bias=col_t[:, C_BGR:C_BGR + 1])
                # bounce out + in + uu
                for h in range(H):
                    nc.sync.dma_start(
                        wgdd[rb, :, :, h, :].rearrange("g m n -> m g n"),
                        wgd_il[h * GM:(h + 1) * GM, :, :])
                wst = wstp.tile([P, H, N], BF16, tag="wst")
                nc.sync.dma_start(
                    wst[:], wgdd[rb].rearrange("g m h n -> (g m) h n"))
                for ob in range(NPAIR):
                    h0 = 2 * ob
                    u_ = wstp.tile([P, 2, N], BF16, tag="u_")
                    nc.gpsimd.tensor_tensor(
                        u_[:], wst[:, h0:h0 + 2, :],
                        objpair[:, rb, None, :].to_broadcast((P, 2, N)),
                        ALU.mult)
                    nc.gpsimd.tensor_tensor(
                        uu_all[:, ob, rb, :, :], u_[:],
                        objc[:, rb, None, :].to_broadcast((P, 2, N)),
                        ALU.add)

        # ---------------- phase 5: attention ----------------
        ot = persist.tile([P, NRB, N], BF16, tag="ot")
        with tc.tile_pool(name="work5", bufs=3) as work5, \
             tc.tile_pool(name="psum5", bufs=2, space="PSUM") as psum5, \
             tc.tile_pool(name="psum_s", bufs=1, space="PSUM") as psum_s, \
             tc.tile_pool(name="psum_av", bufs=1, space="PSUM") as psum_av, \
             tc.tile_pool(name="psum_rb", bufs=1, space="PSUM") as psum_rb:
            for ob in range(NPAIR):
                h0 = 2 * ob
                av = psum_av.tile([P, N], F32, tag="avps")
                sbank = psum_s.tile([2, N], F32, tag="sbank")
                for rb in range(NRB):
                    st2 = psum5.tile([P, 2, N], F32, tag="stps")
                    for hi in range(2):
                        po = hi * DK
                        nc.tensor.matmul(
                            st2[:, hi, :],
                            kTt[po:po + DK, ob, rb * P:(rb + 1) * P],
                            qT[po:po + DK, ob, :], start=True, stop=True)
                    e_ = work5.tile([P, 2, N], BF16, tag="e_t")
                    nc.scalar.activation(e_[:], st2[:], AF.Exp,
                                         bias=col_t[:, C_MC + rb:C_MC + rb + 1])
                    tt_ = work5.tile([P, 2, N], BF16, tag="tt_t")
                    nc.vector.tensor_tensor(
                        tt_[:], e_[:], uu_all[:, ob, rb, :, :], ALU.mult)
                    for hi in range(2):
                        po = hi * DK
                        nc.tensor.matmul(sbank[:], oh2_t[:, hi, :],
                                         tt_[:, hi, :],
                                         start=(rb == 0 and hi == 0),
                                         stop=(rb == NRB - 1 and hi == 1),
                                         skip_group_check=True)
                        nc.tensor.matmul(av[po:po + DK, :],
                                         v_sb[:, rb,
                                              (h0 + hi) * DK:(h0 + hi + 1) * DK],
                                         tt_[:, hi, :], start=(rb == 0),
                                         stop=(rb == NRB - 1),
                                         skip_group_check=True)
                rs = work5.tile([2, N], F32, tag="rs")
                nc.vector.reciprocal(rs[:], sbank[:])
                rrb = psum_rb.tile([P, N], F32, tag="rrb")
                nc.tensor.matmul(rrb[:], selp_f[:], rs[:],
                                 start=True, stop=True)
                av_sb = work5.tile([P, N], F32, tag="av_sb")
                nc.scalar.activation(av_sb[:], av[:], AF.Copy)
                nc.vector.tensor_tensor(ot[:, ob, :], av_sb[:], rrb[:],
                                        ALU.mult)

        # final projection: out[n, d]
        with tc.tile_pool(name="work6", bufs=2) as work6, \
             tc.tile_pool(name="psum6", bufs=2, space="PSUM") as psum6:
            for r in range(NRB):
                ps = psum6.tile([P, D], F32, tag="fps")
                for kt in range(NRB):
                    nc.tensor.matmul(ps[:], ot[:, kt, r * P:(r + 1) * P],
                                     wo_b[:, kt, :],
                                     start=(kt == 0), stop=False)
                nc.tensor.matmul(ps[:], ones_row[0:1, r * P:(r + 1) * P],
                                 brow_t[0:1, BR_O, :], start=False, stop=True)
                fo = work6.tile([P, D], F32, tag="fo")
                nc.scalar.activation(fo[:], ps[:], AF.Copy)
                nc.sync.dma_start(out[r * P:(r + 1) * P, :], fo[:])

    _split_multi_waits(nc)
    return nc


_NC_CACHE = {}


def kernel(**inputs):
    in_maps = _host_prep(inputs)
    if "nc" not in _NC_CACHE:
        _NC_CACHE["nc"] = build_nc()
    nc = _NC_CACHE["nc"]
    res = run_bass_kernel_spmd(nc, in_maps, list(range(B)))
    out = np.stack([res.results[b]["out"] for b in range(B)], axis=0)
    return out.astype(np.float32)


if __name__ == "__main__":
    print("kernel module ok")


# revision 56
# speedup vs baseline: 1.3631x; 1.3631x over previous
"""Trainium2 Bass kernel for BoxMultiHeadedAttention (B=8, N=512, D=512, H=8).

Sharding: data-parallel over batch — each of the 8 NeuronCores computes one
batch element end-to-end; weights replicated; no collectives.

v3 schedule (engine-balanced):
  * host pre-transposes inputs/weights to bf16; all matrix consts are
    host-packed bf16 so no on-chip converts are needed.
  * projections: bias applied via a rank-1 ones-matmul appended to each
    chain; PSUM eviction on ACT (Copy). Chains are interleaved with ph4
    rb-blocks in PE program order to fill PE gaps.
  * ln fields (ph2): ACT Square(+bias)/Ln, Pool subtract/clip; then
    bf16 hi/lo split of dxy2 (ACT copy + Pool subtract) so the phase
    selector runs as 3 bf16 matmuls instead of 1 fp32 matmul.
  * wg (ph4): DVE does the magic-round fold (rr/ff) and the double-angle
    products; ACT does the Sin pair and the Relu eviction
    (wgdR = Relu(wg + bG' - eps), with the -1+eps constant folded into a
    precomputed objc tile); Pool holds lhs_wh (all precomputed early).
  * wgd h-major -> m-major shuffle via DRAM bounce (8 per-h writes + 1
    read per rb); uu = objc + objpair*wgdR computed on Pool per rb.
  * attention (ph5): st2 (PE) -> Exp (ACT) -> tt = e*uu (DVE) ->
    AV/rowsum (PE, accumulating); softmax 1/s broadcast across partitions
    with a one-hot matmul; final linear with bias-matmul + ACT evict.
"""
import math
import numpy as np
from contextlib import ExitStack

import concourse.bass as bass
import concourse.mybir as mybir
import concourse.tile as tile
from concourse.bass_utils import run_bass_kernel_spmd

F32 = mybir.dt.float32
BF16 = mybir.dt.bfloat16
AF = mybir.ActivationFunctionType
ALU = mybir.AluOpType

B, N, D, H = 8, 512, 512, 8
DK = D // H
P = 128
NRB = N // P
NG = 8
GM = 16
NPAIR = H // 2
WAVE_LEN = 1000.0
MAGIC = 12582912.0
C2 = float(2.0 * math.log(0.001))
ESHIFT = -6.0
CM1 = 1e-6 - 1.0
TWO_PI = float(2.0 * math.pi)
HALF_PI = float(math.pi / 2.0)
PI_ = float(math.pi)

_alphas = (100.0 / (WAVE_LEN ** (np.arange(8) / 8.0))).astype(np.float64)

# column indices in colpack
C_BQ, C_BK, C_MC, C_CX, C_CY, C_NCX, C_NCY, C_OC = 0, 4, 8, 12, 16, 20, 24, 28
C_BGR, C_HPI = 32, 33
NCOL = 34
# row indices in rowg
R_CX, R_CY, R_LW, R_LH = range(4)
NROWG = 4
# rows in browpack
BR_Q, BR_K, BR_V, BR_O = range(4)


def _split_multi_waits(nc):
    """walrus here accepts only ONE sync-wait per ISA instruction; hoist
    extras onto NoOps inserted before the offending instruction."""
    n_fix = 0
    for blk in nc.main_func.blocks:
        insts = list(blk.instructions)
        out, dirty = [], False
        for inst in insts:
            si = inst.sync_info
            waits = list(si.on_wait) if si is not None else []
            if len(waits) > 1:
                for kk, w in enumerate(waits[:-1]):
                    out.append(mybir.InstNoOp(
                        name=f"I-waitfix-{n_fix}-{kk}", engine=inst.engine,
                        sync_info=mybir.SyncInfo(on_wait=[w], on_update=[])))
                inst.sync_info = mybir.SyncInfo(
                    on_wait=[waits[-1]], on_update=list(si.on_update))
                n_fix += 1
                dirty = True
            out.append(inst)
        if dirty:
            blk.instructions = out
    return n_fix


def _selector_const():
    # SELAP[64*W + q*16 + m_loc, q, m_loc*8 + j] = alpha_j/(4pi)
    selap = np.zeros((P, 4, P), dtype=np.float32)
    for W in range(2):
        for q in range(4):
            for m_loc in range(GM):
                for j in range(8):
                    selap[64 * W + q * 16 + m_loc, q, m_loc * 8 + j] = \
                        _alphas[j] / (4.0 * math.pi)
    return selap


def _onehot2():
    oh = np.zeros((P, 2, 2), dtype=np.float32)
    for hi in range(2):
        oh[:, hi, hi] = 1.0
    return oh


def _selpair():
    sp = np.zeros((2, P), dtype=np.float32)
    for hi in range(2):
        sp[hi, hi * DK:(hi + 1) * DK] = 1.0
    return sp


def _wg_consts(WG, bG):
    out = {}
    gmap = [lambda j: j, lambda j: 32 + j, lambda j: 8 + j, lambda j: 40 + j]
    gscl = [1.0, -2.0, 1.0, -2.0]
    wblk = np.zeros((4, P, P), dtype=np.float32)
    for c in range(4):
        for m_loc in range(GM):
            for j in range(8):
                for h in range(H):
                    wblk[c, m_loc * 8 + j, h * GM + m_loc] = \
                        gscl[c] * WG[h, gmap[c](j)]
    out["WBLK"] = wblk.transpose(1, 0, 2).copy()  # [P, 4, P]

    acol = np.zeros((64, 1), np.float32)
    pcol_m = np.zeros((64, 1), np.float32)
    pcol_n = np.zeros((64, 1), np.float32)
    w1 = np.zeros((64, H), np.float32)
    for f in range(2):
        for j in range(8):
            gs = 16 + 8 * f + j
            gc = 48 + 8 * f + j
            a = _alphas[j] / (4.0 * math.pi)
            for t in range(4):
                k = (f * 8 + j) * 4 + t
                acol[k, 0] = a
                pcol_m[k, 0] = 0.25 if t in (0, 2) else 0.0
                if t == 0:
                    pcol_n[k, 0] = 0.0; w1[k] = WG[:, gs]
                elif t == 1:
                    pcol_n[k, 0] = 0.75; w1[k] = WG[:, gs]   # -cos -> +pi
                elif t == 2:
                    pcol_n[k, 0] = 0.25; w1[k] = WG[:, gc]
                else:
                    pcol_n[k, 0] = 0.0; w1[k] = WG[:, gc]
    out["ACOL"] = acol
    out["PCOL_M"], out["PCOL_N"] = pcol_m, pcol_n
    out["W1E"] = np.repeat(w1, GM, axis=1).astype(np.float32)
    bg2 = bG.astype(np.float64) + WG[:, 32:48].sum(axis=1)
    out["BGR"] = np.repeat((bg2 - 1e-6).astype(np.float32), GM)
    return out


def _to_bf16(a):
    import ml_dtypes
    return np.asarray(a, np.float32).astype(ml_dtypes.bfloat16)


def _host_prep(inputs):
    q = np.asarray(inputs["input_query"], np.float32)
    k = np.asarray(inputs["input_key"], np.float32)
    v = np.asarray(inputs["input_value"], np.float32)
    box = np.asarray(inputs["input_box"], np.float32)
    mask = np.asarray(inputs["mask"])
    nobj = np.asarray(inputs["not_objects"])
    WG = np.asarray(inputs["WG"], np.float32)
    bG = np.asarray(inputs["bG"], np.float32)
    wgc = _wg_consts(WG, bG)
    sela = _selector_const()
    selah = _to_bf16(sela)
    selal = _to_bf16(sela - np.asarray(selah, np.float32))
    selhl = np.concatenate([np.asarray(selah, np.float32)[:, None],
                            np.asarray(selal, np.float32)[:, None]],
                           axis=1)  # [P, 2, 4, P]

    x_min, y_min, x_max, y_max = [box[..., i] for i in range(4)]
    cx = (x_min + x_max) * 0.5
    cy = (y_min + y_max) * 0.5
    ww = x_max - x_min + 1.0
    hh = y_max - y_min + 1.0
    l2w = (2.0 * np.log(ww)).astype(np.float32)
    l2h = (2.0 * np.log(hh)).astype(np.float32)

    maskcol = (np.where(mask == 0, -1e9, 0.0) + ESHIFT).astype(np.float32)
    obj = (1.0 - nobj.astype(np.float32)).astype(np.float32)

    def wtile(W, scale=1.0):
        return _to_bf16((np.asarray(W, np.float32) * scale)
                        .reshape(NRB, P, D).transpose(1, 0, 2).copy())

    def xtile(x):
        return _to_bf16(x.T.reshape(NRB, P, N).transpose(1, 0, 2).copy())

    w64 = np.zeros((64, 131), np.float32)
    w64[:, :128] = wgc["W1E"]
    w64[:, 128] = wgc["ACOL"][:, 0]
    w64[:, 129] = wgc["PCOL_M"][:, 0]
    w64[:, 130] = wgc["PCOL_N"][:, 0]

    brow = np.zeros((1, 4, D), np.float32)
    brow[0, BR_Q] = np.asarray(inputs["bq"], np.float32)
    brow[0, BR_K] = np.asarray(inputs["bk"], np.float32) * 0.125
    brow[0, BR_V] = np.asarray(inputs["bv"], np.float32)
    brow[0, BR_O] = np.asarray(inputs["bo"], np.float32)

    shared = {
        "Wqb": wtile(inputs["Wq"]),
        "Wkb": wtile(inputs["Wk"], 0.125),
        "Wvb": wtile(inputs["Wv"]),
        "Wob": wtile(inputs["Wo"]),
        "SELAPR": sela,
        "WBLK": _to_bf16(wgc["WBLK"]),
        "W64": w64,
        "OH2": _to_bf16(_onehot2()),
        "NEGI": _to_bf16(-np.eye(P, dtype=np.float32)),
        "SELP": _selpair(),
        "BROW": _to_bf16(brow),
    }
    in_maps = []
    for b in range(B):
        colpack = np.zeros((P, NCOL), np.float32)
        colpack[:, C_BQ:C_BQ + 4] = np.asarray(inputs["bq"], np.float32) \
            .reshape(NRB, P).T
        colpack[:, C_BK:C_BK + 4] = (np.asarray(inputs["bk"], np.float32)
                                     * 0.125).reshape(NRB, P).T
        colpack[:, C_MC:C_MC + 4] = maskcol[b].reshape(NRB, P).T
        colpack[:, C_CX:C_CX + 4] = cx[b].reshape(NRB, P).T
        colpack[:, C_CY:C_CY + 4] = cy[b].reshape(NRB, P).T
        colpack[:, C_NCX:C_NCX + 4] = -cx[b].reshape(NRB, P).T
        colpack[:, C_NCY:C_NCY + 4] = -cy[b].reshape(NRB, P).T
        colpack[:, C_OC:C_OC + 4] = obj[b].reshape(NRB, P).T
        colpack[:, C_BGR] = wgc["BGR"]
        colpack[:, C_HPI] = HALF_PI

        rowg = np.zeros((NROWG, N), np.float32)
        rowg[R_CX] = cx[b]
        rowg[R_CY] = cy[b]
        rowg[R_LW] = l2w[b]
        rowg[R_LH] = l2h[b]

        opm = (obj[b][:, None] * obj[b][None, :]) > 0.5   # [m, n]
        opmask3 = opm.reshape(NRB, P, N).transpose(1, 0, 2)[:, NRB - 1, :] \
            .astype(np.uint8).copy()

        m = dict(shared)
        m.update({
            "xqT": xtile(q[b]), "xkT": xtile(k[b]), "xvT": xtile(v[b]),
            "colpack": colpack, "rowg": rowg,
            "objrow": obj[b].copy(), "opmask3": opmask3,
        })
        in_maps.append(m)
    return in_maps


def build_nc():
    nc = bass.Bass()

    def dp(name, shape, dt=F32):
        return nc.declare_dram_parameter(name, list(shape), dt, isOutput=False)

    colpack = dp("colpack", (P, NCOL))
    rowg = dp("rowg", (NROWG, N))
    objrow = dp("objrow", (N,))
    opmask3 = dp("opmask3", (P, N), mybir.dt.uint8)
    xqT = dp("xqT", (P, NRB, N), BF16)
    xkT = dp("xkT", (P, NRB, N), BF16)
    xvT = dp("xvT", (P, NRB, N), BF16)
    Wqb = dp("Wqb", (P, NRB, D), BF16)
    Wkb = dp("Wkb", (P, NRB, D), BF16)
    Wvb = dp("Wvb", (P, NRB, D), BF16)
    Wob = dp("Wob", (P, NRB, D), BF16)
    SELAPR = dp("SELAPR", (P, 4, P), mybir.dt.float32r)
    WBLK = dp("WBLK", (P, 4, P), BF16)
    W64 = dp("W64", (64, 131))
    OH2 = dp("OH2", (P, 2, 2), BF16)
    NEGI = dp("NEGI", (P, P), BF16)
    SELP = dp("SELP", (2, P), mybir.dt.float32r)
    BROW = dp("BROW", (1, 4, D), BF16)
    out = nc.declare_dram_parameter("out", [N, D], F32, isOutput=True)
    wgdd = nc.dram_tensor("wgdd", [NRB, NG, GM, H, N], BF16)

    with ExitStack() as ctx:
        tc = ctx.enter_context(tile.TileContext(nc))
        const = ctx.enter_context(tc.tile_pool(name="const", bufs=1))
        persist = ctx.enter_context(tc.tile_pool(name="persist", bufs=1))

        # ---------------- loads (critical geo consts first) ----------------
        xv_cm = tc.tile_pool(name="xv", bufs=1)
        xv = xv_cm.__enter__()
        xw_cm = tc.tile_pool(name="xw", bufs=1)
        xw = xw_cm.__enter__()
        col_t = const.tile([P, NCOL], F32, tag="colpk")
        nc.sync.dma_start(col_t[:], colpack[:])
        rowbc = xw.tile([P, NROWG, N], F32, tag="rowg")
        nc.sync.dma_start(
            rowbc[:], rowg[None, :, :].to_broadcast((P, NROWG, N)))
        w64_t = const.tile([64, 131], F32, tag="w64")
        nc.sync.dma_start(w64_t[:], W64[:])
        selap_t = const.tile([P, 4, P], mybir.dt.float32r, tag="selapr")
        nc.sync.dma_start(selap_t[:], SELAPR[:])

        xqb = xw.tile([P, NRB, N], BF16, tag="xqb")
        nc.sync.dma_start(xqb[:], xqT[:])
        wq_b = xw.tile([P, NRB, D], BF16, tag="wqb")
        nc.sync.dma_start(wq_b[:], Wqb[:])
        xkb = xw.tile([P, NRB, N], BF16, tag="xkb")
        nc.sync.dma_start(xkb[:], xkT[:])
        wk_b = xw.tile([P, NRB, D], BF16, tag="wkb")
        nc.sync.dma_start(wk_b[:], Wkb[:])
        xvb = xv.tile([P, NRB, N], BF16, tag="xvb")
        nc.sync.dma_start(xvb[:], xvT[:])
        wv_b = xv.tile([P, NRB, D], BF16, tag="wvb")
        nc.sync.dma_start(wv_b[:], Wvb[:])

        wblk_b = const.tile([P, 4, P], BF16, tag="wblkb")
        nc.sync.dma_start(wblk_b[:], WBLK[:])
        objbc_f = const.tile([P, N], F32, tag="objbcf")
        nc.sync.dma_start(objbc_f[:], objrow[None, :].to_broadcast((P, N)))
        opm3_t = const.tile([P, N], mybir.dt.uint8, tag="opm3")
        nc.sync.dma_start(opm3_t[:], opmask3[:])
        oh2_t = const.tile([P, 2, 2], BF16, tag="oh2")
        nc.sync.dma_start(oh2_t[:], OH2[:])
        negi_t = const.tile([P, P], BF16, tag="negi")
        nc.sync.dma_start(negi_t[:], NEGI[:])
        selp_f = const.tile([2, P], mybir.dt.float32r, tag="selpf")
        nc.sync.dma_start(selp_f[:], SELP[:])
        brow_t = const.tile([1, 4, D], BF16, tag="brow")
        nc.sync.dma_start(brow_t[:], BROW[:])
        wo_b = persist.tile([P, NRB, D], BF16, tag="wob")
        nc.sync.dma_start(wo_b[:], Wob[:])

        ones_row = const.tile([1, N], BF16, tag="ones_row")
        nc.vector.memset(ones_row[:], 1.0)
        objbc = const.tile([P, N], BF16, tag="objbc")
        nc.gpsimd.tensor_copy(objbc[:], objbc_f[:])

        w1e_f = w64_t[:, 0:128]
        acol_t = w64_t[:, 128:129]
        pcolm_t = w64_t[:, 129:130]
        pcoln_t = w64_t[:, 130:131]

        # ---------------- ph2: ln fields + hi/lo split ----------------
        dxyf = persist.tile([P, NRB, 2, N], mybir.dt.float32r, tag="dxyf")
        with tc.tile_pool(name="work2", bufs=2) as work2:
            for rb in range(NRB):
                for (ci, rbc, ncc) in ((0, R_CX, C_NCX), (1, R_CY, C_NCY)):
                    d2 = work2.tile([P, N], F32, tag="geo_d2")
                    nc.scalar.activation(d2[:], rowbc[:, rbc, :], AF.Square,
                                         bias=col_t[:, ncc + rb:ncc + rb + 1])
                    l2t = work2.tile([P, N], F32, tag="geo_l2")
                    nc.scalar.activation(l2t[:], d2[:], AF.Ln)
                    g_ = work2.tile([P, N], F32, tag="geo_g")
                    nc.vector.tensor_tensor(
                        g_[:], l2t[:], rowbc[:, R_LW + ci, :], ALU.subtract)
                    nc.vector.tensor_scalar(dxyf[:, rb, ci, :], g_[:],
                                            C2, None, ALU.max)

        # ---------------- ph3: dw/dh banks (Pool + ACT) ----------------
        bankM = persist.tile([64, N], BF16, tag="bankM")
        bankN = persist.tile([64, N], BF16, tag="bankN")
        with tc.tile_pool(name="work3", bufs=2) as work3:
            for (pcol, bank) in ((pcolm_t, bankM), (pcoln_t, bankN)):
                t_ = work3.tile([64, N], F32, tag="bk_t")
                nc.gpsimd.tensor_scalar(t_[:32, :], rowbc[:32, R_LW, :],
                                        acol_t[:32, :], pcol[:32, :],
                                        ALU.mult, ALU.add)
                nc.gpsimd.tensor_scalar(t_[32:, :], rowbc[32:64, R_LH, :],
                                        acol_t[32:, :], pcol[32:, :],
                                        ALU.mult, ALU.add)
                r_ = work3.tile([64, N], F32, tag="bk_r")
                nc.gpsimd.tensor_scalar(r_[:], t_[:], MAGIC, -MAGIC,
                                        ALU.add, ALU.add)
                f_ = work3.tile([64, N], F32, tag="bk_f")
                nc.gpsimd.tensor_tensor(f_[:], t_[:], r_[:], ALU.subtract)
                nc.scalar.activation(bank[:], f_[:], AF.Sin, scale=TWO_PI)

        # lhs_wh precompute (Pool): [64, rb, g, P]
        lhs_all = persist.tile([64, NRB, NG, P], BF16, tag="lhs_all")
        for rb in range(NRB):
            for g in range(NG):
                mbase = rb * P + g * GM
                nc.gpsimd.tensor_tensor(
                    lhs_all[:, rb, g, :].rearrange("k (h m) -> k h m", h=H),
                    w1e_f.rearrange("k (h m) -> k h m", h=H),
                    bankM[:, mbase:mbase + GM][:, None, :]
                        .to_broadcast((64, H, GM)),
                    ALU.mult)

        # objpair/objc per rb (Pool)
        objpair = persist.tile([P, NRB, N], BF16, tag="objpair")
        objc = persist.tile([P, NRB, N], BF16, tag="objc")
        for rb in range(NRB):
            nc.gpsimd.tensor_scalar(objpair[:, rb, :], objbc[:],
                                    col_t[:, C_OC + rb:C_OC + rb + 1], None,
                                    ALU.mult)
            nc.gpsimd.tensor_scalar(objc[:, rb, :], objpair[:, rb, :],
                                    CM1, 1.0, ALU.mult, ALU.add)

        # ---------------- ph1 + ph4 interleaved on PE ----------------
        qT = persist.tile([P, NRB, N], BF16, tag="qT")
        kTt = persist.tile([P, NRB, N], BF16, tag="kT")
        v_sb = persist.tile([P, NRB, D], BF16, tag="v_sb")
        uu_all = persist.tile([P, NPAIR, NRB, 2, N], BF16, tag="uu_all")
        nc.vector.memset(uu_all[:, :, NRB - 1, :, :], 1.0)

        with tc.tile_pool(name="work4", bufs=3) as work4, \
             tc.tile_pool(name="ilpool", bufs=2) as ilpool, \
             tc.tile_pool(name="wstp", bufs=2) as wstp, \
             tc.tile_pool(name="psum1", bufs=1, space="PSUM") as psum1, \
             tc.tile_pool(name="psum_u", bufs=3, space="PSUM") as psum_u, \
             tc.tile_pool(name="psum_wg", bufs=1, space="PSUM") as psum_wg:

            def qk_chain(ob):
                for (wb_, xb, dstT, bcol) in ((wq_b, xqb, qT, C_BQ),
                                              (wk_b, xkb, kTt, C_BK)):
                    ps = psum1.tile([P, N], F32, tag="projps")
                    for kb in range(NRB):
                        nc.tensor.matmul(ps[:],
                                         wb_[:, kb, ob * P:(ob + 1) * P],
                                         xb[:, kb, :],
                                         start=(kb == 0),
                                         stop=(kb == NRB - 1))
                    nc.vector.tensor_scalar(dstT[:, ob, :], ps[:],
                                            col_t[:, bcol + ob:bcol + ob + 1],
                                            None, ALU.add)

            def v_chain(mb):
                ps = psum1.tile([P, D], F32, tag="projps")
                for kb in range(NRB):
                    nc.tensor.matmul(ps[:], xvb[:, kb, mb * P:(mb + 1) * P],
                                     wv_b[:, kb, :],
                                     start=(kb == 0), stop=False)
                nc.tensor.matmul(ps[:], ones_row[0:1, mb * P:(mb + 1) * P],
                                 brow_t[0:1, BR_V, :], start=False, stop=True)
                nc.scalar.activation(v_sb[:, mb, :], ps[:], AF.Copy)

            it = 0
            for rb in range(NRB):
                if rb == 1:
                    qk_chain(0)
                    qk_chain(1)
                elif rb == 2:
                    qk_chain(2)
                    qk_chain(3)
                elif rb == 3:
                    for mb in range(NRB):
                        v_chain(mb)
                wgd_il = ilpool.tile([P, NG, N], BF16, tag="wgd_il")
                for g in range(NG):
                    ups = psum_u.tile([P, 2, N], F32, tag="ups")
                    off = 64 * (g // 4)
                    qq = g % 4
                    for ci in range(2):
                        nc.tensor.matmul(ups[:, ci, :],
                                         selap_t[off:off + 64, qq, :],
                                         dxyf[off:off + 64, rb, ci, :],
                                         start=True, stop=False)
                    # fold: rr2 = round(t) exactly (integers < 256 are
                    # exact in bf16); subtract it in PSUM via a -I matmul so
                    # the Sin pair reads the folded phase straight from PSUM.
                    rr2 = work4.tile([P, 2, N], BF16, tag="rr2")
                    nc.vector.tensor_scalar(rr2[:], ups[:], MAGIC, -MAGIC,
                                            ALU.add, ALU.add)
                    it += 1
                    for ci in range(2):
                        nc.tensor.matmul(ups[:, ci, :], negi_t[:],
                                         rr2[:, ci, :], start=False,
                                         stop=True, skip_group_check=True)
                    # sin-feature directly: fs = sin(2*pi*f), arg in [-pi,pi)
                    fs = work4.tile([P, 2, N], BF16, tag="fs")
                    nc.scalar.activation(fs[:], ups[:], AF.Sin, scale=TWO_PI)
                    s2 = work4.tile([P, 2, N], BF16, tag="s2")
                    nc.scalar.activation(s2[:], ups[:], AF.Sin, scale=PI_)
                    fcos = work4.tile([P, 2, N], BF16, tag="fcos")
                    nc.vector.tensor_tensor(fcos[:], s2[:], s2[:], ALU.mult)
                    wgp = psum_wg.tile([P, N], F32, tag="wgp")
                    nc.tensor.matmul(wgp[:], wblk_b[:, 0, :], fs[:, 0, :],
                                     start=True, stop=False)
                    nc.tensor.matmul(wgp[:], wblk_b[:, 1, :], fcos[:, 0, :],
                                     start=False, stop=False)
                    nc.tensor.matmul(wgp[:], wblk_b[:, 2, :], fs[:, 1, :],
                                     start=False, stop=False)
                    nc.tensor.matmul(wgp[:], wblk_b[:, 3, :], fcos[:, 1, :],
                                     start=False, stop=False)
                    nc.tensor.matmul(wgp[:], lhs_all[:, rb, g, :], bankN[:],
                                     start=False, stop=True)
                    nc.vector.tensor_scalar(
                        wgd_il[:, g, :], wgp[:],
                        col_t[:, C_BGR:C_BGR + 1], 0.0,
                        ALU.add, ALU.max)
                # bounce out (SP/HWDGE) + in (Pool/SWDGE, dodges the SP
                # queue) + uu. Last rb: per-pair reads + DVE uu (short tail).
                for h in range(H):
                    nc.sync.dma_start(
                        wgdd[rb, :, :, h, :].rearrange("g m n -> m g n"),
                        wgd_il[h * GM:(h + 1) * GM, :, :])
                last = (rb == NRB - 1)
                eng = nc.vector if last else nc.gpsimd
                if last:
                    for ob in range(NPAIR):
                        w2 = wstp.tile([P, 2, N], BF16, tag="u_")
                        nc.gpsimd.dma_start(
                            w2[:], wgdd[rb, :, :, 2 * ob:2 * ob + 2, :]
                            .rearrange("g m h n -> (g m) h n"))
                        nc.vector.copy_predicated(
                            uu_all[:, ob, rb, :, :],
                            opm3_t[:, None, :].to_broadcast((P, 2, N)),
                            w2[:])
                else:
                    wst = wstp.tile([P, H, N], BF16, tag="wst")
                    nc.gpsimd.dma_start(
                        wst[:], wgdd[rb].rearrange("g m h n -> (g m) h n"))
                    for ob in range(NPAIR):
                        u_ = wstp.tile([P, 2, N], BF16, tag="u_")
                        eng.tensor_tensor(
                            u_[:], wst[:, 2 * ob:2 * ob + 2, :],
                            objpair[:, rb, None, :].to_broadcast((P, 2, N)),
                            ALU.mult)
                        eng.tensor_tensor(
                            uu_all[:, ob, rb, :, :], u_[:],
                            objc[:, rb, None, :].to_broadcast((P, 2, N)),
                            ALU.add)

        xw_cm.__exit__(None, None, None)
        xv_cm.__exit__(None, None, None)

        # ---------------- phase 5: attention ----------------
        ot = persist.tile([P, NRB, N], BF16, tag="ot")
        with tc.tile_pool(name="work5", bufs=3) as work5, \
             tc.tile_pool(name="psum5", bufs=2, space="PSUM") as psum5, \
             tc.tile_pool(name="psum_s", bufs=1, space="PSUM") as psum_s, \
             tc.tile_pool(name="psum_av", bufs=1, space="PSUM") as psum_av, \
             tc.tile_pool(name="psum_rb", bufs=1, space="PSUM") as psum_rb:
            for ob in range(NPAIR):
                h0 = 2 * ob
                av = psum_av.tile([P, N], F32, tag="avps")
                sbank = psum_s.tile([2, N], F32, tag="sbank")
                for rb in range(NRB):
                    st2 = psum5.tile([P, 2, N], F32, tag="stps")
                    for hi in range(2):
                        po = hi * DK
                        nc.tensor.matmul(
                            st2[:, hi, :],
                            kTt[po:po + DK, ob, rb * P:(rb + 1) * P],
                            qT[po:po + DK, ob, :], start=True, stop=True)
                    e_ = work5.tile([P, 2, N], BF16, tag="e_t")
                    nc.scalar.activation(e_[:], st2[:], AF.Exp,
                                         bias=col_t[:, C_MC + rb:C_MC + rb + 1])
                    tt_ = work5.tile([P, 2, N], BF16, tag="tt_t")
                    nc.vector.tensor_tensor(
                        tt_[:], e_[:], uu_all[:, ob, rb, :, :], ALU.mult)
                    for hi in range(2):
                        po = hi * DK
                        nc.tensor.matmul(sbank[:], oh2_t[:, hi, :],
                                         tt_[:, hi, :],
                                         start=(rb == 0 and hi == 0),
                                         stop=(rb == NRB - 1 and hi == 1),
                                         skip_group_check=True)
                        nc.tensor.matmul(av[po:po + DK, :],
                                         v_sb[:, rb,
                                              (h0 + hi) * DK:(h0 + hi + 1) * DK],
                                         tt_[:, hi, :], start=(rb == 0),
                                         stop=(rb == NRB - 1),
                                         skip_group_check=True)
                rs = work5.tile([2, N], mybir.dt.float32r, tag="rs")
                with nc.allow_low_precision(reason="f32r recip broadcast"):
                    nc.vector.reciprocal(rs[:], sbank[:])
                rrb = psum_rb.tile([P, N], F32, tag="rrb")
                nc.tensor.matmul(rrb[:], selp_f[:], rs[:],
                                 start=True, stop=True)
                av_sb = work5.tile([P, N], F32, tag="av_sb")
                nc.scalar.activation(av_sb[:], av[:], AF.Copy)
                nc.vector.tensor_tensor(ot[:, ob, :], av_sb[:], rrb[:],
                                        ALU.mult)

        # final projection: out[n, d]
        with tc.tile_pool(name="work6", bufs=2) as work6, \
             tc.tile_pool(name="psum6", bufs=2, space="PSUM") as psum6:
            for r in range(NRB):
                ps = psum6.tile([P, D], F32, tag="fps")
                for kt in range(NRB):
                    nc.tensor.matmul(ps[:], ot[:, kt, r * P:(r + 1) * P],
                                     wo_b[:, kt, :],
                                     start=(kt == 0), stop=False)
                nc.tensor.matmul(ps[:], ones_row[0:1, r * P:(r + 1) * P],
                                 brow_t[0:1, BR_O, :], start=False, stop=True)
                fo = work6.tile([P, D], F32, tag="fo")
                nc.scalar.activation(fo[:], ps[:], AF.Copy)
                nc.sync.dma_start(out[r * P:(r + 1) * P, :], fo[:])

    _split_multi_waits(nc)
    return nc


_NC_CACHE = {}


def kernel(**inputs):
    in_maps = _host_prep(inputs)
    if "nc" not in _NC_CACHE:
        _NC_CACHE["nc"] = build_nc()
    nc = _NC_CACHE["nc"]
    res = run_bass_kernel_spmd(nc, in_maps, list(range(B)))
    out = np.stack([res.results[b]["out"] for b in range(B)], axis=0)
    return out.astype(np.float32)


if __name__ == "__main__":
    print("kernel module ok")


# revision 57
# speedup vs baseline: 1.3842x; 1.0155x over previous
"""Trainium2 Bass kernel for BoxMultiHeadedAttention (B=8, N=512, D=512, H=8).

Sharding: data-parallel over batch — each of the 8 NeuronCores computes one
batch element end-to-end; weights replicated; no collectives.

v3 schedule (engine-balanced):
  * host pre-transposes inputs/weights to bf16; all matrix consts are
    host-packed bf16 so no on-chip converts are needed.
  * projections: bias applied via a rank-1 ones-matmul appended to each
    chain; PSUM eviction on ACT (Copy). Chains are interleaved with ph4
    rb-blocks in PE program order to fill PE gaps.
  * ln fields (ph2): ACT Square(+bias)/Ln, Pool subtract/clip; then
    bf16 hi/lo split of dxy2 (ACT copy + Pool subtract) so the phase
    selector runs as 3 bf16 matmuls instead of 1 fp32 matmul.
  * wg (ph4): DVE does the magic-round fold (rr/ff) and the double-angle
    products; ACT does the Sin pair and the Relu eviction
    (wgdR = Relu(wg + bG' - eps), with the -1+eps constant folded into a
    precomputed objc tile); Pool holds lhs_wh (all precomputed early).
  * wgd h-major -> m-major shuffle via DRAM bounce (8 per-h writes + 1
    read per rb); uu = objc + objpair*wgdR computed on Pool per rb.
  * attention (ph5): st2 (PE) -> Exp (ACT) -> tt = e*uu (DVE) ->
    AV/rowsum (PE, accumulating); softmax 1/s broadcast across partitions
    with a one-hot matmul; final linear with bias-matmul + ACT evict.
"""
import math
import numpy as np
from contextlib import ExitStack

import concourse.bass as bass
import concourse.mybir as mybir
import concourse.tile as tile
from concourse.bass_utils import run_bass_kernel_spmd

F32 = mybir.dt.float32
BF16 = mybir.dt.bfloat16
AF = mybir.ActivationFunctionType
ALU = mybir.AluOpType

B, N, D, H = 8, 512, 512, 8
DK = D // H
P = 128
NRB = N // P
NG = 8
GM = 16
NPAIR = H // 2
WAVE_LEN = 1000.0
MAGIC = 12582912.0
C2 = float(2.0 * math.log(0.001))
ESHIFT = -6.0
CM1 = 1e-6 - 1.0
TWO_PI = float(2.0 * math.pi)
HALF_PI = float(math.pi / 2.0)
PI_ = float(math.pi)

_alphas = (100.0 / (WAVE_LEN ** (np.arange(8) / 8.0))).astype(np.float64)

# column indices in colpack
C_BQ, C_BK, C_MC, C_CX, C_CY, C_NCX, C_NCY, C_OC = 0, 4, 8, 12, 16, 20, 24, 28
C_BGR, C_HPI = 32, 33
NCOL = 34
# row indices in rowg
R_CX, R_CY, R_LW, R_LH = range(4)
NROWG = 4
# rows in browpack
BR_Q, BR_K, BR_V, BR_O = range(4)


def _split_multi_waits(nc):
    """walrus here accepts only ONE sync-wait per ISA instruction; hoist
    extras onto NoOps inserted before the offending instruction."""
    n_fix = 0
    for blk in nc.main_func.blocks:
        insts = list(blk.instructions)
        out, dirty = [], False
        for inst in insts:
            si = inst.sync_info
            waits = list(si.on_wait) if si is not None else []
            if len(waits) > 1:
                for kk, w in enumerate(waits[:-1]):
                    out.append(mybir.InstNoOp(
                        name=f"I-waitfix-{n_fix}-{kk}", engine=inst.engine,
                        sync_info=mybir.SyncInfo(on_wait=[w], on_update=[])))
                inst.sync_info = mybir.SyncInfo(
                    on_wait=[waits[-1]], on_update=list(si.on_update))
                n_fix += 1
                dirty = True
            out.append(inst)
        if dirty:
            blk.instructions = out
    return n_fix


def _selector_const():
    # SELAP[64*W + q*16 + m_loc, q, m_loc*8 + j] = alpha_j/(4pi)
    selap = np.zeros((P, 4, P), dtype=np.float32)
    for W in range(2):
        for q in range(4):
            for m_loc in range(GM):
                for j in range(8):
                    selap[64 * W + q * 16 + m_loc, q, m_loc * 8 + j] = \
                        _alphas[j] / (4.0 * math.pi)
    return selap


def _onehot2():
    oh = np.zeros((P, 2, 2), dtype=np.float32)
    for hi in range(2):
        oh[:, hi, hi] = 1.0
    return oh


def _selpair():
    sp = np.zeros((2, P), dtype=np.float32)
    for hi in range(2):
        sp[hi, hi * DK:(hi + 1) * DK] = 1.0
    return sp


def _wg_consts(WG, bG):
    out = {}
    gmap = [lambda j: j, lambda j: 32 + j, lambda j: 8 + j, lambda j: 40 + j]
    gscl = [1.0, -2.0, 1.0, -2.0]
    wblk = np.zeros((4, P, P), dtype=np.float32)
    for c in range(4):
        for m_loc in range(GM):
            for j in range(8):
                for h in range(H):
                    wblk[c, m_loc * 8 + j, h * GM + m_loc] = \
                        gscl[c] * WG[h, gmap[c](j)]
    out["WBLK"] = wblk.transpose(1, 0, 2).copy()  # [P, 4, P]

    acol = np.zeros((64, 1), np.float32)
    pcol_m = np.zeros((64, 1), np.float32)
    pcol_n = np.zeros((64, 1), np.float32)
    w1 = np.zeros((64, H), np.float32)
    for f in range(2):
        for j in range(8):
            gs = 16 + 8 * f + j
            gc = 48 + 8 * f + j
            a = _alphas[j] / (4.0 * math.pi)
            for t in range(4):
                k = (f * 8 + j) * 4 + t
                acol[k, 0] = a
                pcol_m[k, 0] = 0.25 if t in (0, 2) else 0.0
                if t == 0:
                    pcol_n[k, 0] = 0.0; w1[k] = WG[:, gs]
                elif t == 1:
                    pcol_n[k, 0] = 0.75; w1[k] = WG[:, gs]   # -cos -> +pi
                elif t == 2:
                    pcol_n[k, 0] = 0.25; w1[k] = WG[:, gc]
                else:
                    pcol_n[k, 0] = 0.0; w1[k] = WG[:, gc]
    out["ACOL"] = acol
    out["PCOL_M"], out["PCOL_N"] = pcol_m, pcol_n
    out["W1E"] = np.repeat(w1, GM, axis=1).astype(np.float32)
    bg2 = bG.astype(np.float64) + WG[:, 32:48].sum(axis=1)
    out["BGR"] = np.repeat((bg2 - 1e-6).astype(np.float32), GM)
    return out


def _to_bf16(a):
    import ml_dtypes
    return np.asarray(a, np.float32).astype(ml_dtypes.bfloat16)


def _host_prep(inputs):
    q = np.asarray(inputs["input_query"], np.float32)
    k = np.asarray(inputs["input_key"], np.float32)
    v = np.asarray(inputs["input_value"], np.float32)
    box = np.asarray(inputs["input_box"], np.float32)
    mask = np.asarray(inputs["mask"])
    nobj = np.asarray(inputs["not_objects"])
    WG = np.asarray(inputs["WG"], np.float32)
    bG = np.asarray(inputs["bG"], np.float32)
    wgc = _wg_consts(WG, bG)
    sela = _selector_const()
    selah = _to_bf16(sela)
    selal = _to_bf16(sela - np.asarray(selah, np.float32))
    selhl = np.concatenate([np.asarray(selah, np.float32)[:, None],
                            np.asarray(selal, np.float32)[:, None]],
                           axis=1)  # [P, 2, 4, P]

    x_min, y_min, x_max, y_max = [box[..., i] for i in range(4)]
    cx = (x_min + x_max) * 0.5
    cy = (y_min + y_max) * 0.5
    ww = x_max - x_min + 1.0
    hh = y_max - y_min + 1.0
    l2w = (2.0 * np.log(ww)).astype(np.float32)
    l2h = (2.0 * np.log(hh)).astype(np.float32)

    maskcol = (np.where(mask == 0, -1e9, 0.0) + ESHIFT).astype(np.float32)
    obj = (1.0 - nobj.astype(np.float32)).astype(np.float32)

    def wtile(W, scale=1.0):
        return _to_bf16((np.asarray(W, np.float32) * scale)
                        .reshape(NRB, P, D).transpose(1, 0, 2).copy())

    def xtile(x):
        return _to_bf16(x.T.reshape(NRB, P, N).transpose(1, 0, 2).copy())

    w64 = np.zeros((64, 131), np.float32)
    w64[:, :128] = wgc["W1E"]
    w64[:, 128] = wgc["ACOL"][:, 0]
    w64[:, 129] = wgc["PCOL_M"][:, 0]
    w64[:, 130] = wgc["PCOL_N"][:, 0]

    brow = np.zeros((1, 4, D), np.float32)
    brow[0, BR_Q] = np.asarray(inputs["bq"], np.float32)
    brow[0, BR_K] = np.asarray(inputs["bk"], np.float32) * 0.125
    brow[0, BR_V] = np.asarray(inputs["bv"], np.float32)
    brow[0, BR_O] = np.asarray(inputs["bo"], np.float32)

    shared = {
        "Wqb": wtile(inputs["Wq"]),
        "Wkb": wtile(inputs["Wk"], 0.125),
        "Wvb": wtile(inputs["Wv"]),
        "Wob": wtile(inputs["Wo"]),
        "SELAPR": sela,
        "WBLK": _to_bf16(wgc["WBLK"]),
        "W64": w64,
        "OH2": _to_bf16(_onehot2()),
        "NEGI": _to_bf16(-np.eye(P, dtype=np.float32)),
        "SELP": _selpair(),
        "BROW": _to_bf16(brow),
    }
    in_maps = []
    for b in range(B):
        colpack = np.zeros((P, NCOL), np.float32)
        colpack[:, C_BQ:C_BQ + 4] = np.asarray(inputs["bq"], np.float32) \
            .reshape(NRB, P).T
        colpack[:, C_BK:C_BK + 4] = (np.asarray(inputs["bk"], np.float32)
                                     * 0.125).reshape(NRB, P).T
        colpack[:, C_MC:C_MC + 4] = maskcol[b].reshape(NRB, P).T
        colpack[:, C_CX:C_CX + 4] = cx[b].reshape(NRB, P).T
        colpack[:, C_CY:C_CY + 4] = cy[b].reshape(NRB, P).T
        colpack[:, C_NCX:C_NCX + 4] = -cx[b].reshape(NRB, P).T
        colpack[:, C_NCY:C_NCY + 4] = -cy[b].reshape(NRB, P).T
        colpack[:, C_OC:C_OC + 4] = obj[b].reshape(NRB, P).T
        colpack[:, C_BGR] = wgc["BGR"]
        colpack[:, C_HPI] = HALF_PI

        rowg = np.zeros((NROWG, N), np.float32)
        rowg[R_CX] = cx[b]
        rowg[R_CY] = cy[b]
        rowg[R_LW] = l2w[b]
        rowg[R_LH] = l2h[b]

        m = dict(shared)
        m.update({
            "xqT": xtile(q[b]), "xkT": xtile(k[b]), "xvT": xtile(v[b]),
            "colpack": colpack, "rowg": rowg,
            "objrow": obj[b].copy(),
        })
        in_maps.append(m)
    return in_maps


def build_nc():
    nc = bass.Bass()

    def dp(name, shape, dt=F32):
        return nc.declare_dram_parameter(name, list(shape), dt, isOutput=False)

    colpack = dp("colpack", (P, NCOL))
    rowg = dp("rowg", (NROWG, N))
    objrow = dp("objrow", (N,))
    xqT = dp("xqT", (P, NRB, N), BF16)
    xkT = dp("xkT", (P, NRB, N), BF16)
    xvT = dp("xvT", (P, NRB, N), BF16)
    Wqb = dp("Wqb", (P, NRB, D), BF16)
    Wkb = dp("Wkb", (P, NRB, D), BF16)
    Wvb = dp("Wvb", (P, NRB, D), BF16)
    Wob = dp("Wob", (P, NRB, D), BF16)
    SELAPR = dp("SELAPR", (P, 4, P), mybir.dt.float32r)
    WBLK = dp("WBLK", (P, 4, P), BF16)
    W64 = dp("W64", (64, 131))
    OH2 = dp("OH2", (P, 2, 2), BF16)
    NEGI = dp("NEGI", (P, P), BF16)
    SELP = dp("SELP", (2, P), mybir.dt.float32r)
    BROW = dp("BROW", (1, 4, D), BF16)
    out = nc.declare_dram_parameter("out", [N, D], F32, isOutput=True)
    wgdd = nc.dram_tensor("wgdd", [NRB, NG, GM, H, N], BF16)

    with ExitStack() as ctx:
        tc = ctx.enter_context(tile.TileContext(nc))
        const = ctx.enter_context(tc.tile_pool(name="const", bufs=1))
        persist = ctx.enter_context(tc.tile_pool(name="persist", bufs=1))

        # ---------------- loads (critical geo consts first) ----------------
        xv_cm = tc.tile_pool(name="xv", bufs=1)
        xv = xv_cm.__enter__()
        xw_cm = tc.tile_pool(name="xw", bufs=1)
        xw = xw_cm.__enter__()
        col_t = const.tile([P, NCOL], F32, tag="colpk")
        nc.sync.dma_start(col_t[:], colpack[:])
        rowbc = xw.tile([P, NROWG, N], F32, tag="rowg")
        nc.sync.dma_start(
            rowbc[:], rowg[None, :, :].to_broadcast((P, NROWG, N)))
        w64_t = const.tile([64, 131], F32, tag="w64")
        nc.sync.dma_start(w64_t[:], W64[:])
        selap_t = const.tile([P, 4, P], mybir.dt.float32r, tag="selapr")
        nc.sync.dma_start(selap_t[:], SELAPR[:])

        xqb = xw.tile([P, NRB, N], BF16, tag="xqb")
        nc.sync.dma_start(xqb[:], xqT[:])
        wq_b = xw.tile([P, NRB, D], BF16, tag="wqb")
        nc.sync.dma_start(wq_b[:], Wqb[:])
        xkb = xw.tile([P, NRB, N], BF16, tag="xkb")
        nc.sync.dma_start(xkb[:], xkT[:])
        wk_b = xw.tile([P, NRB, D], BF16, tag="wkb")
        nc.sync.dma_start(wk_b[:], Wkb[:])
        xvb = xv.tile([P, NRB, N], BF16, tag="xvb")
        nc.sync.dma_start(xvb[:], xvT[:])
        wv_b = xv.tile([P, NRB, D], BF16, tag="wvb")
        nc.sync.dma_start(wv_b[:], Wvb[:])

        wblk_b = const.tile([P, 4, P], BF16, tag="wblkb")
        nc.sync.dma_start(wblk_b[:], WBLK[:])
        objbc_f = const.tile([P, N], F32, tag="objbcf")
        nc.sync.dma_start(objbc_f[:], objrow[None, :].to_broadcast((P, N)))
        oh2_t = const.tile([P, 2, 2], BF16, tag="oh2")
        nc.sync.dma_start(oh2_t[:], OH2[:])
        negi_t = const.tile([P, P], BF16, tag="negi")
        nc.sync.dma_start(negi_t[:], NEGI[:])
        selp_f = const.tile([2, P], mybir.dt.float32r, tag="selpf")
        nc.sync.dma_start(selp_f[:], SELP[:])
        brow_t = const.tile([1, 4, D], BF16, tag="brow")
        nc.sync.dma_start(brow_t[:], BROW[:])
        wo_b = persist.tile([P, NRB, D], BF16, tag="wob")
        nc.sync.dma_start(wo_b[:], Wob[:])

        ones_row = const.tile([1, N], BF16, tag="ones_row")
        nc.vector.memset(ones_row[:], 1.0)
        objbc = const.tile([P, N], BF16, tag="objbc")
        nc.gpsimd.tensor_copy(objbc[:], objbc_f[:])

        w1e_f = w64_t[:, 0:128]
        acol_t = w64_t[:, 128:129]
        pcolm_t = w64_t[:, 129:130]
        pcoln_t = w64_t[:, 130:131]

        # ---------------- ph2: ln fields + hi/lo split ----------------
        dxyf = persist.tile([P, NRB, 2, N], mybir.dt.float32r, tag="dxyf")
        with tc.tile_pool(name="work2", bufs=2) as work2:
            for rb in range(NRB):
                for (ci, rbc, ncc) in ((0, R_CX, C_NCX), (1, R_CY, C_NCY)):
                    d2 = work2.tile([P, N], F32, tag="geo_d2")
                    nc.scalar.activation(d2[:], rowbc[:, rbc, :], AF.Square,
                                         bias=col_t[:, ncc + rb:ncc + rb + 1])
                    l2t = work2.tile([P, N], F32, tag="geo_l2")
                    nc.scalar.activation(l2t[:], d2[:], AF.Ln)
                    g_ = work2.tile([P, N], F32, tag="geo_g")
                    nc.vector.tensor_tensor(
                        g_[:], l2t[:], rowbc[:, R_LW + ci, :], ALU.subtract)
                    nc.vector.tensor_scalar(dxyf[:, rb, ci, :], g_[:],
                                            C2, None, ALU.max)

        # ---------------- ph3: dw/dh banks (Pool + ACT) ----------------
        bankM = persist.tile([64, N], BF16, tag="bankM")
        bankN = persist.tile([64, N], BF16, tag="bankN")
        with tc.tile_pool(name="work3", bufs=2) as work3:
            for (pcol, bank) in ((pcolm_t, bankM), (pcoln_t, bankN)):
                t_ = work3.tile([64, N], F32, tag="bk_t")
                nc.gpsimd.tensor_scalar(t_[:32, :], rowbc[:32, R_LW, :],
                                        acol_t[:32, :], pcol[:32, :],
                                        ALU.mult, ALU.add)
                nc.gpsimd.tensor_scalar(t_[32:, :], rowbc[32:64, R_LH, :],
                                        acol_t[32:, :], pcol[32:, :],
                                        ALU.mult, ALU.add)
                r_ = work3.tile([64, N], F32, tag="bk_r")
                nc.gpsimd.tensor_scalar(r_[:], t_[:], MAGIC, -MAGIC,
                                        ALU.add, ALU.add)
                f_ = work3.tile([64, N], F32, tag="bk_f")
                nc.gpsimd.tensor_tensor(f_[:], t_[:], r_[:], ALU.subtract)
                nc.scalar.activation(bank[:], f_[:], AF.Sin, scale=TWO_PI)

        # lhs_wh precompute (Pool): [64, rb, g, P]
        lhs_all = persist.tile([64, NRB, NG, P], BF16, tag="lhs_all")
        for rb in range(NRB):
            for g in range(NG):
                mbase = rb * P + g * GM
                nc.gpsimd.tensor_tensor(
                    lhs_all[:, rb, g, :].rearrange("k (h m) -> k h m", h=H),
                    w1e_f.rearrange("k (h m) -> k h m", h=H),
                    bankM[:, mbase:mbase + GM][:, None, :]
                        .to_broadcast((64, H, GM)),
                    ALU.mult)

        # objpair/objc per rb (Pool)
        objpair = persist.tile([P, NRB, N], BF16, tag="objpair")
        objc = persist.tile([P, NRB, N], BF16, tag="objc")
        for rb in range(NRB):
            nc.gpsimd.tensor_scalar(objpair[:, rb, :], objbc[:],
                                    col_t[:, C_OC + rb:C_OC + rb + 1], None,
                                    ALU.mult)
            nc.gpsimd.tensor_scalar(objc[:, rb, :], objpair[:, rb, :],
                                    CM1, 1.0, ALU.mult, ALU.add)

        # ---------------- ph1 + ph4 interleaved on PE ----------------
        qT = persist.tile([P, NRB, N], BF16, tag="qT")
        kTt = persist.tile([P, NRB, N], BF16, tag="kT")
        v_sb = persist.tile([P, NRB, D], BF16, tag="v_sb")
        uu_all = persist.tile([P, NPAIR, NRB, 2, N], BF16, tag="uu_all")

        with tc.tile_pool(name="work4", bufs=3) as work4, \
             tc.tile_pool(name="ilpool", bufs=2) as ilpool, \
             tc.tile_pool(name="wstp", bufs=2) as wstp, \
             tc.tile_pool(name="psum1", bufs=1, space="PSUM") as psum1, \
             tc.tile_pool(name="psum_u", bufs=3, space="PSUM") as psum_u, \
             tc.tile_pool(name="psum_wg", bufs=1, space="PSUM") as psum_wg:

            def qk_chain(ob):
                for (wb_, xb, dstT, bcol) in ((wq_b, xqb, qT, C_BQ),
                                              (wk_b, xkb, kTt, C_BK)):
                    ps = psum1.tile([P, N], F32, tag="projps")
                    for kb in range(NRB):
                        nc.tensor.matmul(ps[:],
                                         wb_[:, kb, ob * P:(ob + 1) * P],
                                         xb[:, kb, :],
                                         start=(kb == 0),
                                         stop=(kb == NRB - 1))
                    nc.vector.tensor_scalar(dstT[:, ob, :], ps[:],
                                            col_t[:, bcol + ob:bcol + ob + 1],
                                            None, ALU.add)

            def v_chain(mb):
                ps = psum1.tile([P, D], F32, tag="projps")
                for kb in range(NRB):
                    nc.tensor.matmul(ps[:], xvb[:, kb, mb * P:(mb + 1) * P],
                                     wv_b[:, kb, :],
                                     start=(kb == 0), stop=False)
                nc.tensor.matmul(ps[:], ones_row[0:1, mb * P:(mb + 1) * P],
                                 brow_t[0:1, BR_V, :], start=False, stop=True)
                nc.scalar.activation(v_sb[:, mb, :], ps[:], AF.Copy)

            it = 0
            for rb in range(NRB):
                if rb == 1:
                    qk_chain(0)
                    qk_chain(1)
                elif rb == 2:
                    qk_chain(2)
                    qk_chain(3)
                elif rb == 3:
                    for mb in range(NRB):
                        v_chain(mb)
                wgd_il = ilpool.tile([P, NG, N], BF16, tag="wgd_il")
                for g in range(NG):
                    ups = psum_u.tile([P, 2, N], F32, tag="ups")
                    off = 64 * (g // 4)
                    qq = g % 4
                    for ci in range(2):
                        nc.tensor.matmul(ups[:, ci, :],
                                         selap_t[off:off + 64, qq, :],
                                         dxyf[off:off + 64, rb, ci, :],
                                         start=True, stop=False)
                    # fold: rr2 = round(t) exactly (integers < 256 are
                    # exact in bf16); subtract it in PSUM via a -I matmul so
                    # the Sin pair reads the folded phase straight from PSUM.
                    rr2 = work4.tile([P, 2, N], BF16, tag="rr2")
                    nc.vector.tensor_scalar(rr2[:], ups[:], MAGIC, -MAGIC,
                                            ALU.add, ALU.add)
                    it += 1
                    for ci in range(2):
                        nc.tensor.matmul(ups[:, ci, :], negi_t[:],
                                         rr2[:, ci, :], start=False,
                                         stop=True, skip_group_check=True)
                    # sin-feature directly: fs = sin(2*pi*f), arg in [-pi,pi)
                    fs = work4.tile([P, 2, N], BF16, tag="fs")
                    nc.scalar.activation(fs[:], ups[:], AF.Sin, scale=TWO_PI)
                    s2 = work4.tile([P, 2, N], BF16, tag="s2")
                    nc.scalar.activation(s2[:], ups[:], AF.Sin, scale=PI_)
                    fcos = work4.tile([P, 2, N], BF16, tag="fcos")
                    nc.vector.tensor_tensor(fcos[:], s2[:], s2[:], ALU.mult)
                    wgp = psum_wg.tile([P, N], F32, tag="wgp")
                    nc.tensor.matmul(wgp[:], wblk_b[:, 0, :], fs[:, 0, :],
                                     start=True, stop=False)
                    nc.tensor.matmul(wgp[:], wblk_b[:, 1, :], fcos[:, 0, :],
                                     start=False, stop=False)
                    nc.tensor.matmul(wgp[:], wblk_b[:, 2, :], fs[:, 1, :],
                                     start=False, stop=False)
                    nc.tensor.matmul(wgp[:], wblk_b[:, 3, :], fcos[:, 1, :],
                                     start=False, stop=False)
                    nc.tensor.matmul(wgp[:], lhs_all[:, rb, g, :], bankN[:],
                                     start=False, stop=True)
                    nc.vector.tensor_scalar(
                        wgd_il[:, g, :], wgp[:],
                        col_t[:, C_BGR:C_BGR + 1], 0.0,
                        ALU.add, ALU.max)
                # bounce out (SP/HWDGE) + in (Pool/SWDGE, dodges the SP
                # queue) + uu. Last rb: per-pair reads + DVE uu (short tail).
                for h in range(H):
                    nc.sync.dma_start(
                        wgdd[rb, :, :, h, :].rearrange("g m n -> m g n"),
                        wgd_il[h * GM:(h + 1) * GM, :, :])
                last = (rb == NRB - 1)
                eng = nc.vector if last else nc.gpsimd
                if last:
                    for ob in range(NPAIR):
                        w2 = wstp.tile([P, 2, N], BF16, tag="u_")
                        nc.gpsimd.dma_start(
                            w2[:], wgdd[rb, :, :, 2 * ob:2 * ob + 2, :]
                            .rearrange("g m h n -> (g m) h n"))
                        u_ = wstp.tile([P, 2, N], BF16, tag="u_")
                        eng.tensor_tensor(
                            u_[:], w2[:],
                            objpair[:, rb, None, :].to_broadcast((P, 2, N)),
                            ALU.mult)
                        eng.tensor_tensor(
                            uu_all[:, ob, rb, :, :], u_[:],
                            objc[:, rb, None, :].to_broadcast((P, 2, N)),
                            ALU.add)
                else:
                    wst = wstp.tile([P, H, N], BF16, tag="wst")
                    nc.gpsimd.dma_start(
                        wst[:], wgdd[rb].rearrange("g m h n -> (g m) h n"))
                    for ob in range(NPAIR):
                        u_ = wstp.tile([P, 2, N], BF16, tag="u_")
                        eng.tensor_tensor(
                            u_[:], wst[:, 2 * ob:2 * ob + 2, :],
                            objpair[:, rb, None, :].to_broadcast((P, 2, N)),
                            ALU.mult)
                        eng.tensor_tensor(
                            uu_all[:, ob, rb, :, :], u_[:],
                            objc[:, rb, None, :].to_broadcast((P, 2, N)),
                            ALU.add)

        xw_cm.__exit__(None, None, None)
        xv_cm.__exit__(None, None, None)

        # ---------------- phase 5: attention ----------------
        ot = persist.tile([P, NRB, N], BF16, tag="ot")
        with tc.tile_pool(name="work5", bufs=3) as work5, \
             tc.tile_pool(name="psum5", bufs=2, space="PSUM") as psum5, \
             tc.tile_pool(name="psum_s", bufs=1, space="PSUM") as psum_s, \
             tc.tile_pool(name="psum_av", bufs=1, space="PSUM") as psum_av, \
             tc.tile_pool(name="psum_rb", bufs=1, space="PSUM") as psum_rb:
            for ob in range(NPAIR):
                h0 = 2 * ob
                av = psum_av.tile([P, N], F32, tag="avps")
                sbank = psum_s.tile([2, N], F32, tag="sbank")
                for rb in range(NRB):
                    st2 = psum5.tile([P, 2, N], F32, tag="stps")
                    for hi in range(2):
                        po = hi * DK
                        nc.tensor.matmul(
                            st2[:, hi, :],
                            kTt[po:po + DK, ob, rb * P:(rb + 1) * P],
                            qT[po:po + DK, ob, :], start=True, stop=True)
                    e_ = work5.tile([P, 2, N], BF16, tag="e_t")
                    nc.scalar.activation(e_[:], st2[:], AF.Exp,
                                         bias=col_t[:, C_MC + rb:C_MC + rb + 1])
                    tt_ = work5.tile([P, 2, N], BF16, tag="tt_t")
                    nc.vector.tensor_tensor(
                        tt_[:], e_[:], uu_all[:, ob, rb, :, :], ALU.mult)
                    for hi in range(2):
                        po = hi * DK
                        nc.tensor.matmul(sbank[:], oh2_t[:, hi, :],
                                         tt_[:, hi, :],
                                         start=(rb == 0 and hi == 0),
                                         stop=(rb == NRB - 1 and hi == 1),
                                         skip_group_check=True)
                        nc.tensor.matmul(av[po:po + DK, :],
                                         v_sb[:, rb,
                                              (h0 + hi) * DK:(h0 + hi + 1) * DK],
                                         tt_[:, hi, :], start=(rb == 0),
                                         stop=(rb == NRB - 1),
                                         skip_group_check=True)
                rs = work5.tile([2, N], mybir.dt.float32r, tag="rs")
                with nc.allow_low_precision(reason="f32r recip broadcast"):
                    nc.vector.reciprocal(rs[:], sbank[:])
                rrb = psum_rb.tile([P, N], F32, tag="rrb")
                nc.tensor.matmul(rrb[:], selp_f[:], rs[:],
                                 start=True, stop=True)
                av_sb = work5.tile([P, N], F32, tag="av_sb")
                nc.scalar.activation(av_sb[:], av[:], AF.Copy)
                nc.vector.tensor_tensor(ot[:, ob, :], av_sb[:], rrb[:],
                                        ALU.mult)

        # final projection: out[n, d]
        with tc.tile_pool(name="work6", bufs=2) as work6, \
             tc.tile_pool(name="psum6", bufs=2, space="PSUM") as psum6:
            for r in range(NRB):
                ps = psum6.tile([P, D], F32, tag="fps")
                for kt in range(NRB):
                    nc.tensor.matmul(ps[:], ot[:, kt, r * P:(r + 1) * P],
                                     wo_b[:, kt, :],
                                     start=(kt == 0), stop=False)
                nc.tensor.matmul(ps[:], ones_row[0:1, r * P:(r + 1) * P],
                                 brow_t[0:1, BR_O, :], start=False, stop=True)
                fo = work6.tile([P, D], F32, tag="fo")
                nc.scalar.activation(fo[:], ps[:], AF.Copy)
                nc.sync.dma_start(out[r * P:(r + 1) * P, :], fo[:])

    _split_multi_waits(nc)
    return nc


_NC_CACHE = {}


def kernel(**inputs):
    in_maps = _host_prep(inputs)
    if "nc" not in _NC_CACHE:
        _NC_CACHE["nc"] = build_nc()
    nc = _NC_CACHE["nc"]
    res = run_bass_kernel_spmd(nc, in_maps, list(range(B)))
    out = np.stack([res.results[b]["out"] for b in range(B)], axis=0)
    return out.astype(np.float32)


if __name__ == "__main__":
    print("kernel module ok")
